# revision 18
# baseline (speedup 1.0000x reference)
"""Fused full-attention kernel for Trainium2, SPMD over 8 NeuronCores.

Problem: nn_CausalSelfAttention (B=4, T=2048, D=1024, H=16, head_dim=64),
with the module's faithful-to-torch raw `.view(3,B,T,D)` reinterpretation of
the (B,T,3D) QKV projection buffer (NOT a feature-dim chunk), full (non-causal)
softmax over keys.

Sharding: core c handles batch b=c//2 and head-group hg=c%2 (8 heads). The raw
view means q/k/v token rows map to proj rows n//3 with column-chunk n%3; tokens
are processed in residue-class order (t mod 3), which makes every extraction a
contiguous slice. The host pre-permutes W_qkv columns per (b,hg,class) and
slices x rows per class, so one canonical SPMD program serves all cores. The
final output projection is computed per-core on the head-group's 512 columns;
host sums the two partial outputs per batch, un-permutes rows, and adds b_out.

v3: all operands bf16, uniform 16x128 token tiles, ones-column softmax
denominator, software-pipelined attention inner loop (QK of tile g+1 is
emitted before AV of tile g so the PE never blocks on the exp), and
tile-granular injection of the q-projection (later classes) and the output
projection into the ACT-bound attention phase.

NOTE: b_qkv is compiled in as zero (the problem spec fixes fill=zeros for it).
"""

import numpy as np
import ml_dtypes

import concourse.mybir as mybir
from concourse import bacc
from concourse.bass_utils import run_bass_kernel_spmd
from concourse.tile import TileContext

F32 = mybir.dt.float32
BF16 = mybir.dt.bfloat16
Exp = mybir.ActivationFunctionType.Exp

B, T, D = 4, 2048, 1024
CNT = (683, 683, 682)  # tokens per residue class (t % 3 == j)
OFF = (0, 683, 1366)
NG = 16  # uniform 128-token tiles over the grouped token axis


def build(reps: int = 1, stage: int = 4):
    nc = bacc.Bacc("TRN2", target_bir_lowering=False, debug=False)

    xq = nc.dram_tensor("xq", (D, 2048), BF16, kind="ExternalInput")
    xk = nc.dram_tensor("xk", (D, 2048), BF16, kind="ExternalInput")
    xv = nc.dram_tensor("xv", (D, 2048), BF16, kind="ExternalInput")
    wq = nc.dram_tensor("wq", (D, 1536), BF16, kind="ExternalInput")
    wk = nc.dram_tensor("wk", (D, 1536), BF16, kind="ExternalInput")
    wv = nc.dram_tensor("wv", (D, 1536), BF16, kind="ExternalInput")
    wo = nc.dram_tensor("wo", (512, 1024), BF16, kind="ExternalInput")
    out = nc.dram_tensor("out", (2048, 1024), BF16, kind="ExternalOutput")

    with TileContext(nc) as tc:
        with (
            tc.tile_pool(name="pers", bufs=1) as pers,
            tc.tile_pool(name="g1x", bufs=26) as xp,
            tc.tile_pool(name="g1w", bufs=9) as wp,
            tc.tile_pool(name="g1wv", bufs=10) as wvp,
            tc.tile_pool(name="mm", bufs=2, space="PSUM") as mmp,
            tc.tile_pool(name="att_st", bufs=2, space="PSUM") as stp,
            tc.tile_pool(name="att_in", bufs=1, space="PSUM") as inp,
            tc.tile_pool(name="att_ex", bufs=8) as exp_,
            tc.tile_pool(name="att_sm", bufs=2) as smp,
            tc.tile_pool(name="op_o", bufs=3) as otp,
        ):
            qTs = [pers.tile([128, 2048], BF16, tag=f"qT{i}", name=f"qT{i}") for i in range(4)]
            kTs = [pers.tile([128, 2048], BF16, tag=f"kT{i}", name=f"kT{i}") for i in range(4)]
            vs = [pers.tile([128, 520], BF16, tag=f"v{g}", name=f"v{g}") for g in range(NG)]
            inTs = [pers.tile([128, 2048], BF16, tag=f"inT{i}", name=f"inT{i}") for i in range(4)]
            wos = [pers.tile([128, 1024], BF16, tag=f"wo{i}", name=f"wo{i}") for i in range(4)]

            # ones columns of v (softmax denominator trick): written once,
            # disjoint from the per-rep data columns.
            for g in range(NG):
                vr = vs[g].rearrange("p (h e) -> p h e", e=65)
                nc.vector.memset(vr[:, :, 64:65], 1.0)

            def load_x(xd, j, eng=None):
                eng = eng or nc.scalar
                xts = []
                for d in range(8):
                    xt = xp.tile([128, 704], BF16, tag="x", name="xt")
                    eng.dma_start(
                        xt[:, 0 : CNT[j]],
                        xd[d * 128 : (d + 1) * 128, OFF[j] : OFF[j] + CNT[j]],
                    )
                    xts.append(xt)
                return xts

            def qk_chunk(xts, wt, dst, j, fp, a0, an):
                ps = mmp.tile([128, 512], F32, tag="mm", name="ps")
                for d in range(8):
                    nc.tensor.matmul(
                        ps[:, 0:an],
                        wt[:, d * 128 : (d + 1) * 128],
                        xts[d][:, a0 : a0 + an],
                        start=(d == 0),
                        stop=(d == 7),
                    )
                nc.vector.tensor_copy(
                    dst[fp][:, OFF[j] + a0 : OFF[j] + a0 + an], ps[:, 0:an]
                )

            def load_w(wd, j, fp, eng=None):
                eng = eng or nc.sync
                wd_r = wd.rearrange("(dt p) c -> p dt c", p=128)
                wt = wp.tile([128, 1024], BF16, tag="w", name="wt")
                c0 = j * 512 + fp * 128
                eng.dma_start(
                    wt[:].rearrange("p (dt c) -> p dt c", c=128),
                    wd_r[:, :, c0 : c0 + 128],
                )
                return wt

            def chunks_of(j):
                return (
                    ((0, 384), (CNT[j] - 300, 300))
                    if CNT[j] % 2
                    else ((0, 384), (384, CNT[j] - 384))
                )

            def emit_k():
                xts_all = [load_x(xk, j) for j in range(3)]
                wts_all = [[load_w(wk, j, fp) for fp in range(4)] for j in range(2)]
                for j in range(3):
                    for fp in range(4):
                        wt = wts_all[j][fp] if j < 2 else load_w(wk, j, fp)
                        for a0, an in chunks_of(j):
                            qk_chunk(xts_all[j], wt, kTs, j, fp, a0, an)

            def emit_v():
                # natural layout [tok, f]; realigned into uniform 128-token
                # vs tiles (interleaved 65-stride with the ones columns) by
                # partition-shifting DMAs from an SBUF staging tile.
                for j in range(3):
                    xts = load_x(xv, j)
                    wvts = []
                    for d in range(8):
                        wvt = wvp.tile([128, 512], BF16, tag="wv", name="wvt")
                        nc.scalar.dma_start(
                            wvt[:],
                            wv[d * 128 : (d + 1) * 128, j * 512 : (j + 1) * 512],
                        )
                        wvts.append(wvt)
                    for i0 in range(0, CNT[j], 128):
                        tp = min(128, CNT[j] - i0)
                        ps = mmp.tile([128, 512], F32, tag="mm", name="ps")
                        for d in range(8):
                            nc.tensor.matmul(
                                ps[0:tp, :],
                                xts[d][:, i0 : i0 + tp],
                                wvts[d][:],
                                start=(d == 0),
                                stop=(d == 7),
                            )
                        vst = wvp.tile([128, 512], BF16, tag="vst", name="vst")
                        nc.vector.tensor_copy(vst[0:tp, :], ps[0:tp, :])
                        vsr = vst.rearrange("p (h e) -> p h e", e=64)
                        g0 = OFF[j] + i0
                        g, p0 = g0 // 128, g0 % 128
                        r1 = min(tp, 128 - p0)
                        vr = vs[g].rearrange("p (h e) -> p h e", e=65)
                        nc.sync.dma_start(vr[p0 : p0 + r1, :, 0:64], vsr[0:r1, :, :])
                        if tp > r1:
                            vr2 = vs[g + 1].rearrange("p (h e) -> p h e", e=65)
                            nc.sync.dma_start(
                                vr2[0 : tp - r1, :, 0:64], vsr[r1:tp, :, :]
                            )

            def q_pieces(j):
                # 8 small closures (fp x chunk) emitting the q-projection for
                # class j; x and w tiles are loaded lazily by the first use.
                state = {}

                def piece(fp, a0, an):
                    if "xts" not in state:
                        state["xts"] = load_x(xq, j, eng=nc.sync)
                    if fp not in state:
                        state[fp] = load_w(wq, j, fp, eng=nc.sync)
                    qk_chunk(state["xts"], state[fp], qTs, j, fp, a0, an)

                return [
                    (lambda fp=fp, a0=a0, an=an: piece(fp, a0, an))
                    for fp in range(4)
                    for a0, an in chunks_of(j)
                ]

            def load_wo():
                for i in range(4):
                    nc.sync.dma_start(wos[i][:], wo[i * 128 : (i + 1) * 128, :])

            def op_fused(s16):
                # one row-block of the output projection: each stationary
                # (inT slice) is loaded once and serves both 512-col halves,
                # halving the LDWEIGHTS traffic.
                ot = otp.tile([128, 1024], BF16, tag="ot", name="ot")
                psA = mmp.tile([128, 512], F32, tag="mm", name="psA")
                psB = mmp.tile([128, 512], F32, tag="mm", name="psB")
                for fp in range(4):
                    nc.tensor.matmul(
                        psA[:],
                        inTs[fp][:, s16 * 128 : (s16 + 1) * 128],
                        wos[fp][:, 0:512],
                        start=(fp == 0),
                        stop=(fp == 3),
                    )
                    nc.tensor.matmul(
                        psB[:],
                        inTs[fp][:, s16 * 128 : (s16 + 1) * 128],
                        wos[fp][:, 512:1024],
                        start=(fp == 0),
                        stop=(fp == 3),
                    )
                nc.vector.tensor_copy(ot[:, 0:512], psA[:])
                nc.vector.tensor_copy(ot[:, 512:1024], psB[:])
                oeng = nc.scalar if s16 % 2 else nc.sync
                oeng.dma_start(out[s16 * 128 : (s16 + 1) * 128, :], ot[:])

            def attn_segment(s, fp, inject):
                sc0 = s * 512
                hA, hB = 2 * fp, 2 * fp + 1

                def qk(g):
                    t0 = g * 128
                    st = stp.tile([128, 1024], F32, tag="st", name="st")
                    nc.tensor.matmul(
                        st[:, 0:512],
                        kTs[fp][0:64, t0 : t0 + 128],
                        qTs[fp][0:64, sc0 : sc0 + 512],
                        start=True, stop=True, tile_position=(0, 0),
                    )
                    nc.tensor.matmul(
                        st[:, 512:1024],
                        kTs[fp][64:128, t0 : t0 + 128],
                        qTs[fp][64:128, sc0 : sc0 + 512],
                        start=True, stop=True, tile_position=(64, 0),
                    )
                    return st

                if stage >= 3:
                    inA = inp.tile([65, 512], F32, tag="inA", name="inA")
                    inB = inp.tile([65, 512], F32, tag="inB", name="inB")
                st = qk(0)
                for g in range(NG):
                    if stage >= 2:
                        ex = exp_.tile([128, 1024], BF16, tag="ex", name="ex")
                        nc.scalar.activation(ex[:], st[:], Exp, scale=0.125)
                    if g + 1 < NG:
                        st = qk(g + 1)
                    if stage >= 3:
                        nc.tensor.matmul(
                            inA[:],
                            vs[g][:, hA * 65 : hA * 65 + 65],
                            ex[:, 0:512],
                            start=(g == 0), stop=(g == NG - 1),
                        )
                        nc.tensor.matmul(
                            inB[:],
                            vs[g][:, hB * 65 : hB * 65 + 65],
                            ex[:, 512:1024],
                            start=(g == 0), stop=(g == NG - 1),
                        )
                    cb = inject.get(g)
                    if cb is not None:
                        cb()
                if stage < 3:
                    return
                # copy PSUM accumulators to SBUF promptly (frees the banks for
                # the next segment), then normalize by the ones-row sums.
                sAB = smp.tile([65, 1024], F32, tag="sAB", name="sAB")
                nc.vector.tensor_copy(sAB[:, 0:512], inA[:])
                nc.vector.tensor_copy(sAB[:, 512:1024], inB[:])
                rec = smp.tile([1, 1024], F32, tag="rec", name="rec")
                nc.vector.reciprocal(rec[:], sAB[64:65, :])
                bc = smp.tile([64, 1024], F32, tag="bc", name="bc")
                nc.gpsimd.partition_broadcast(bc[:], rec[:])
                nc.vector.tensor_mul(
                    inTs[fp][0:64, sc0 : sc0 + 512], sAB[0:64, 0:512], bc[:, 0:512]
                )
                stB = smp.tile([64, 512], BF16, tag="stB", name="stB")
                nc.vector.tensor_mul(stB[:], sAB[0:64, 512:1024], bc[:, 512:1024])
                nc.sync.dma_start(inTs[fp][64:128, sc0 : sc0 + 512], stB[:])

            def body():
                if stage >= 4:
                    load_wo()
                emit_k()
                emit_v()
                for u in q_pieces(0):
                    u()
                if stage == 0:
                    for j in (1, 2):
                        for u in q_pieces(j):
                            u()
                    return

                # injection schedule: {(s, fp): {tile g: closure}}
                inject = {(s, fp): {} for s in range(4) for fp in range(4)}

                def spread(s, items, slots):
                    n = len(items) // 4
                    for fp in range(4):
                        part = items[fp * n : (fp + 1) * n]
                        for i, it in enumerate(part):
                            inject[(s, fp)][slots[i]] = it

                spread(0, q_pieces(1), (5, 11))
                spread(1, q_pieces(2), (5, 11))
                if stage >= 4:
                    # output-projection row blocks: OP for sblk s is injected
                    # one sblk later (its inTs columns are complete by then);
                    # sblk3's own rows must wait for the tail.
                    for fp in range(4):
                        inject[(1, fp)][8] = lambda i=fp: op_fused(i)
                        inject[(2, fp)][8] = lambda i=4 + fp: op_fused(i)
                        inject[(3, fp)][8] = lambda i=8 + fp: op_fused(i)

                for s in range(4):
                    for fp in range(4):
                        attn_segment(s, fp, inject[(s, fp)])
                if stage >= 4:
                    for s16 in range(12, 16):
                        op_fused(s16)

            if reps == 1:
                body()
            else:
                # device-side loop: one emitted copy of the body, executed
                # `reps` times — amplifies body time for host-side timing
                # without blowing up the instruction count.
                with tc.For_i(0, reps, 1):
                    body()

    nc.compile()
    return nc


_CACHE: dict = {}


def get_nc(reps: int = 1):
    if reps not in _CACHE:
        _CACHE[reps] = build(reps)
    return _CACHE[reps]


_STAGES = {"gemm1": 0, "qk": 1, "exp": 2, "av": 3, "full": 4}


def build_mode(reps: int, mode: str):
    return build(reps, _STAGES[mode])


def shard_inputs(x, W_qkv, W_out):
    bf16 = ml_dtypes.bfloat16
    xf = np.ascontiguousarray(np.asarray(x, dtype=np.float32)).reshape(B * T, D)
    W_qkv = np.asarray(W_qkv, dtype=np.float32)
    W_out = np.asarray(W_out, dtype=np.float32)
    per_core = []
    for c in range(8):
        b, hg = c // 2, c % 2
        XQ = np.zeros((2048, D), np.float32)
        XK = np.zeros((2048, D), np.float32)
        XV = np.zeros((2048, D), np.float32)
        WQ = np.zeros((D, 1536), np.float32)
        WK = np.zeros((D, 1536), np.float32)
        WV = np.zeros((D, 1536), np.float32)
        for j in range(3):
            cnt, off = CNT[j], OFF[j]
            for XX, WW, base in (
                (XQ, WQ, b * 2048 + j),
                (XK, WK, 8192 + b * 2048 + j),
                (XV, WV, 16384 + b * 2048 + j),
            ):
                r0, ch = base // 3, base % 3
                XX[off : off + cnt] = xf[r0 : r0 + cnt]
                WW[:, j * 512 : (j + 1) * 512] = W_qkv[
                    :, ch * 1024 + hg * 512 : ch * 1024 + hg * 512 + 512
                ]
        per_core.append(
            dict(
                xq=np.ascontiguousarray(XQ.T).astype(bf16),
                xk=np.ascontiguousarray(XK.T).astype(bf16),
                xv=np.ascontiguousarray(XV.T).astype(bf16),
                wq=WQ.astype(bf16), wk=WK.astype(bf16), wv=WV.astype(bf16),
                wo=np.ascontiguousarray(W_out[hg * 512 : (hg + 1) * 512]).astype(
                    bf16
                ),
            )
        )
    return per_core


_PI = np.concatenate([np.arange(j, 2048, 3) for j in range(3)])


def unshard(core_outs, b_out):
    b_out = np.asarray(b_out, dtype=np.float32)
    out = np.empty((B, T, D), np.float32)
    for b in range(B):
        part = np.asarray(core_outs[2 * b], np.float32) + np.asarray(
            core_outs[2 * b + 1], np.float32
        )
        tmp = np.empty_like(part)
        tmp[_PI] = part
        out[b] = tmp + b_out
    return out


def kernel(x, W_qkv, b_qkv, W_out, b_out, num_heads):
    assert int(num_heads) == 16
    nc = get_nc(1)
    in_maps = shard_inputs(x, W_qkv, W_out)
    res = run_bass_kernel_spmd(nc, in_maps, core_ids=list(range(8)))
    return unshard([r["out"] for r in res.results], b_out)


# revision 19
# speedup vs baseline: 1.0283x; 1.0283x over previous
"""Fused full-attention kernel for Trainium2, SPMD over 8 NeuronCores.

Problem: nn_CausalSelfAttention (B=4, T=2048, D=1024, H=16, head_dim=64),
with the module's faithful-to-torch raw `.view(3,B,T,D)` reinterpretation of
the (B,T,3D) QKV projection buffer (NOT a feature-dim chunk), full (non-causal)
softmax over keys.

Sharding: core c handles batch b=c//2 and head-group hg=c%2 (8 heads). The raw
view means q/k/v token rows map to proj rows n//3 with column-chunk n%3; tokens
are processed in residue-class order (t mod 3), which makes every extraction a
contiguous slice. The host pre-permutes W_qkv columns per (b,hg,class) and
slices x rows per class, so one canonical SPMD program serves all cores. The
final output projection is computed per-core on the head-group's 512 columns;
host sums the two partial outputs per batch, un-permutes rows, and adds b_out.

v3: all operands bf16, uniform 16x128 token tiles, ones-column softmax
denominator, software-pipelined attention inner loop (QK of tile g+1 is
emitted before AV of tile g so the PE never blocks on the exp), and
tile-granular injection of the q-projection (later classes) and the output
projection into the ACT-bound attention phase.

NOTE: b_qkv is compiled in as zero (the problem spec fixes fill=zeros for it).
"""

import numpy as np
import ml_dtypes

import concourse.mybir as mybir
from concourse import bacc
from concourse.bass_utils import run_bass_kernel_spmd
from concourse.tile import TileContext

F32 = mybir.dt.float32
BF16 = mybir.dt.bfloat16
Exp = mybir.ActivationFunctionType.Exp

B, T, D = 4, 2048, 1024
CNT = (683, 683, 682)  # tokens per residue class (t % 3 == j)
OFF = (0, 683, 1366)
NG = 16  # uniform 128-token tiles over the grouped token axis


def build(reps: int = 1, stage: int = 4):
    nc = bacc.Bacc("TRN2", target_bir_lowering=False, debug=False)

    xq = nc.dram_tensor("xq", (D, 2048), BF16, kind="ExternalInput")
    xk = nc.dram_tensor("xk", (D, 2048), BF16, kind="ExternalInput")
    xv = nc.dram_tensor("xv", (D, 2048), BF16, kind="ExternalInput")
    wq = nc.dram_tensor("wq", (D, 1536), BF16, kind="ExternalInput")
    wk = nc.dram_tensor("wk", (D, 1536), BF16, kind="ExternalInput")
    wv = nc.dram_tensor("wv", (D, 1536), BF16, kind="ExternalInput")
    wo = nc.dram_tensor("wo", (512, 1024), BF16, kind="ExternalInput")
    out = nc.dram_tensor("out", (2048, 1024), BF16, kind="ExternalOutput")

    with TileContext(nc) as tc:
        with (
            tc.tile_pool(name="pers", bufs=1) as pers,
            tc.tile_pool(name="g1x", bufs=26) as xp,
            tc.tile_pool(name="g1w", bufs=9) as wp,
            tc.tile_pool(name="g1wv", bufs=10) as wvp,
            tc.tile_pool(name="mm", bufs=2, space="PSUM") as mmp,
            tc.tile_pool(name="att_st", bufs=2, space="PSUM") as stp,
            tc.tile_pool(name="att_in", bufs=1, space="PSUM") as inp,
            tc.tile_pool(name="att_ex", bufs=8) as exp_,
            tc.tile_pool(name="att_sm", bufs=2) as smp,
            tc.tile_pool(name="op_o", bufs=3) as otp,
        ):
            qTs = [pers.tile([128, 2048], BF16, tag=f"qT{i}", name=f"qT{i}") for i in range(4)]
            kTs = [pers.tile([128, 2048], BF16, tag=f"kT{i}", name=f"kT{i}") for i in range(4)]
            vs = [pers.tile([128, 520], BF16, tag=f"v{g}", name=f"v{g}") for g in range(NG)]
            inTs = [pers.tile([128, 2048], BF16, tag=f"inT{i}", name=f"inT{i}") for i in range(4)]
            wos = [pers.tile([128, 1024], BF16, tag=f"wo{i}", name=f"wo{i}") for i in range(4)]

            # ones columns of v (softmax denominator trick): written once,
            # disjoint from the per-rep data columns.
            for g in range(NG):
                vr = vs[g].rearrange("p (h e) -> p h e", e=65)
                nc.vector.memset(vr[:, :, 64:65], 1.0)

            def load_x(xd, j, eng=None):
                eng = eng or nc.scalar
                xts = []
                for d in range(8):
                    xt = xp.tile([128, 704], BF16, tag="x", name="xt")
                    eng.dma_start(
                        xt[:, 0 : CNT[j]],
                        xd[d * 128 : (d + 1) * 128, OFF[j] : OFF[j] + CNT[j]],
                    )
                    xts.append(xt)
                return xts

            def qk_chunk(xts, wt, dst, j, fp, a0, an):
                ps = mmp.tile([128, 512], F32, tag="mm", name="ps")
                for d in range(8):
                    nc.tensor.matmul(
                        ps[:, 0:an],
                        wt[:, d * 128 : (d + 1) * 128],
                        xts[d][:, a0 : a0 + an],
                        start=(d == 0),
                        stop=(d == 7),
                    )
                nc.vector.tensor_copy(
                    dst[fp][:, OFF[j] + a0 : OFF[j] + a0 + an], ps[:, 0:an]
                )

            def load_w(wd, j, fp, eng=None):
                eng = eng or nc.sync
                wd_r = wd.rearrange("(dt p) c -> p dt c", p=128)
                wt = wp.tile([128, 1024], BF16, tag="w", name="wt")
                c0 = j * 512 + fp * 128
                eng.dma_start(
                    wt[:].rearrange("p (dt c) -> p dt c", c=128),
                    wd_r[:, :, c0 : c0 + 128],
                )
                return wt

            def chunks_of(j):
                return (
                    ((0, 384), (CNT[j] - 300, 300))
                    if CNT[j] % 2
                    else ((0, 384), (384, CNT[j] - 384))
                )

            def emit_k():
                xts_all = [load_x(xk, j) for j in range(3)]
                wts_all = [[load_w(wk, j, fp) for fp in range(4)] for j in range(2)]
                for j in range(3):
                    for fp in range(4):
                        wt = wts_all[j][fp] if j < 2 else load_w(wk, j, fp)
                        for a0, an in chunks_of(j):
                            qk_chunk(xts_all[j], wt, kTs, j, fp, a0, an)

            def emit_v():
                # natural layout [tok, f]; realigned into uniform 128-token
                # vs tiles (interleaved 65-stride with the ones columns) by
                # partition-shifting DMAs from an SBUF staging tile.
                for j in range(3):
                    xts = load_x(xv, j)
                    wvts = []
                    for d in range(8):
                        wvt = wvp.tile([128, 512], BF16, tag="wv", name="wvt")
                        nc.scalar.dma_start(
                            wvt[:],
                            wv[d * 128 : (d + 1) * 128, j * 512 : (j + 1) * 512],
                        )
                        wvts.append(wvt)
                    for i0 in range(0, CNT[j], 128):
                        tp = min(128, CNT[j] - i0)
                        ps = mmp.tile([128, 512], F32, tag="mm", name="ps")
                        for d in range(8):
                            nc.tensor.matmul(
                                ps[0:tp, :],
                                xts[d][:, i0 : i0 + tp],
                                wvts[d][:],
                                start=(d == 0),
                                stop=(d == 7),
                            )
                        vst = wvp.tile([128, 512], BF16, tag="vst", name="vst")
                        nc.vector.tensor_copy(vst[0:tp, :], ps[0:tp, :])
                        vsr = vst.rearrange("p (h e) -> p h e", e=64)
                        g0 = OFF[j] + i0
                        g, p0 = g0 // 128, g0 % 128
                        r1 = min(tp, 128 - p0)
                        vr = vs[g].rearrange("p (h e) -> p h e", e=65)
                        nc.sync.dma_start(vr[p0 : p0 + r1, :, 0:64], vsr[0:r1, :, :])
                        if tp > r1:
                            vr2 = vs[g + 1].rearrange("p (h e) -> p h e", e=65)
                            nc.sync.dma_start(
                                vr2[0 : tp - r1, :, 0:64], vsr[r1:tp, :, :]
                            )

            def q_pieces(j):
                # 8 small closures (fp x chunk) emitting the q-projection for
                # class j; x and w tiles are loaded lazily by the first use.
                state = {}

                def piece(fp, a0, an):
                    if "xts" not in state:
                        state["xts"] = load_x(xq, j, eng=nc.sync)
                    if fp not in state:
                        state[fp] = load_w(wq, j, fp, eng=nc.sync)
                    qk_chunk(state["xts"], state[fp], qTs, j, fp, a0, an)

                return [
                    (lambda fp=fp, a0=a0, an=an: piece(fp, a0, an))
                    for fp in range(4)
                    for a0, an in chunks_of(j)
                ]

            def load_wo():
                for i in range(4):
                    nc.sync.dma_start(wos[i][:], wo[i * 128 : (i + 1) * 128, :])

            def op_fused(s16):
                # one row-block of the output projection: each stationary
                # (inT slice) is loaded once and serves both 512-col halves,
                # halving the LDWEIGHTS traffic.
                ot = otp.tile([128, 1024], BF16, tag="ot", name="ot")
                psA = mmp.tile([128, 512], F32, tag="mm", name="psA")
                psB = mmp.tile([128, 512], F32, tag="mm", name="psB")
                for fp in range(4):
                    nc.tensor.matmul(
                        psA[:],
                        inTs[fp][:, s16 * 128 : (s16 + 1) * 128],
                        wos[fp][:, 0:512],
                        start=(fp == 0),
                        stop=(fp == 3),
                    )
                    nc.tensor.matmul(
                        psB[:],
                        inTs[fp][:, s16 * 128 : (s16 + 1) * 128],
                        wos[fp][:, 512:1024],
                        start=(fp == 0),
                        stop=(fp == 3),
                    )
                nc.vector.tensor_copy(ot[:, 0:512], psA[:])
                nc.vector.tensor_copy(ot[:, 512:1024], psB[:])
                nc.sync.dma_start(out[s16 * 128 : (s16 + 1) * 128, :], ot[:])

            def attn_segment(s, fp, inject):
                sc0 = s * 512
                hA, hB = 2 * fp, 2 * fp + 1

                def qk(g):
                    t0 = g * 128
                    st = stp.tile([128, 1024], F32, tag="st", name="st")
                    nc.tensor.matmul(
                        st[:, 0:512],
                        kTs[fp][0:64, t0 : t0 + 128],
                        qTs[fp][0:64, sc0 : sc0 + 512],
                        start=True, stop=True, tile_position=(0, 0),
                    )
                    nc.tensor.matmul(
                        st[:, 512:1024],
                        kTs[fp][64:128, t0 : t0 + 128],
                        qTs[fp][64:128, sc0 : sc0 + 512],
                        start=True, stop=True, tile_position=(64, 0),
                    )
                    return st

                if stage >= 3:
                    inA = inp.tile([65, 512], F32, tag="inA", name="inA")
                    inB = inp.tile([65, 512], F32, tag="inB", name="inB")
                st = qk(0)
                for g in range(NG):
                    if stage >= 2:
                        ex = exp_.tile([128, 1024], BF16, tag="ex", name="ex")
                        nc.scalar.activation(ex[:], st[:], Exp, scale=0.125)
                    if g + 1 < NG:
                        st = qk(g + 1)
                    if stage >= 3:
                        nc.tensor.matmul(
                            inA[:],
                            vs[g][:, hA * 65 : hA * 65 + 65],
                            ex[:, 0:512],
                            start=(g == 0), stop=(g == NG - 1),
                        )
                        nc.tensor.matmul(
                            inB[:],
                            vs[g][:, hB * 65 : hB * 65 + 65],
                            ex[:, 512:1024],
                            start=(g == 0), stop=(g == NG - 1),
                        )
                    cb = inject.get(g)
                    if cb is not None:
                        cb()
                if stage < 3:
                    return
                # copy PSUM accumulators to SBUF promptly (frees the banks for
                # the next segment), then normalize by the ones-row sums.
                sAB = smp.tile([65, 1024], F32, tag="sAB", name="sAB")
                nc.vector.tensor_copy(sAB[:, 0:512], inA[:])
                nc.vector.tensor_copy(sAB[:, 512:1024], inB[:])
                rec = smp.tile([1, 1024], F32, tag="rec", name="rec")
                nc.vector.reciprocal(rec[:], sAB[64:65, :])
                bc = smp.tile([64, 1024], F32, tag="bc", name="bc")
                nc.gpsimd.partition_broadcast(bc[:], rec[:])
                nc.vector.tensor_mul(
                    inTs[fp][0:64, sc0 : sc0 + 512], sAB[0:64, 0:512], bc[:, 0:512]
                )
                stB = smp.tile([64, 512], BF16, tag="stB", name="stB")
                nc.vector.tensor_mul(stB[:], sAB[0:64, 512:1024], bc[:, 512:1024])
                nc.sync.dma_start(inTs[fp][64:128, sc0 : sc0 + 512], stB[:])

            def body():
                if stage >= 4:
                    load_wo()
                emit_k()
                emit_v()
                for u in q_pieces(0):
                    u()
                if stage == 0:
                    for j in (1, 2):
                        for u in q_pieces(j):
                            u()
                    return

                # injection schedule: {(s, fp): {tile g: closure}}
                inject = {(s, fp): {} for s in range(4) for fp in range(4)}

                def spread(s, items, slots):
                    n = len(items) // 4
                    for fp in range(4):
                        part = items[fp * n : (fp + 1) * n]
                        for i, it in enumerate(part):
                            inject[(s, fp)][slots[i]] = it

                spread(0, q_pieces(1), (5, 11))
                spread(1, q_pieces(2), (5, 11))
                if stage >= 4:
                    # output-projection row blocks: 2 fused pieces per sblk2
                    # segment (q rows of sblk0+sblk1), 1 per sblk3 segment,
                    # remainder in the tail.
                    for fp in range(4):
                        inject[(2, fp)][4] = lambda i=fp: op_fused(i)
                        inject[(2, fp)][11] = lambda i=4 + fp: op_fused(i)
                        inject[(3, fp)][7] = lambda i=8 + fp: op_fused(i)

                for s in range(4):
                    for fp in range(4):
                        attn_segment(s, fp, inject[(s, fp)])
                if stage >= 4:
                    for s16 in range(12, 16):
                        op_fused(s16)

            if reps == 1:
                body()
            else:
                # device-side loop: one emitted copy of the body, executed
                # `reps` times — amplifies body time for host-side timing
                # without blowing up the instruction count.
                with tc.For_i(0, reps, 1):
                    body()

    nc.compile()
    return nc


_CACHE: dict = {}


def get_nc(reps: int = 1):
    if reps not in _CACHE:
        _CACHE[reps] = build(reps)
    return _CACHE[reps]


_STAGES = {"gemm1": 0, "qk": 1, "exp": 2, "av": 3, "full": 4}


def build_mode(reps: int, mode: str):
    return build(reps, _STAGES[mode])


def shard_inputs(x, W_qkv, W_out):
    bf16 = ml_dtypes.bfloat16
    xf = np.ascontiguousarray(np.asarray(x, dtype=np.float32)).reshape(B * T, D)
    W_qkv = np.asarray(W_qkv, dtype=np.float32)
    W_out = np.asarray(W_out, dtype=np.float32)
    per_core = []
    for c in range(8):
        b, hg = c // 2, c % 2
        XQ = np.zeros((2048, D), np.float32)
        XK = np.zeros((2048, D), np.float32)
        XV = np.zeros((2048, D), np.float32)
        WQ = np.zeros((D, 1536), np.float32)
        WK = np.zeros((D, 1536), np.float32)
        WV = np.zeros((D, 1536), np.float32)
        for j in range(3):
            cnt, off = CNT[j], OFF[j]
            for XX, WW, base in (
                (XQ, WQ, b * 2048 + j),
                (XK, WK, 8192 + b * 2048 + j),
                (XV, WV, 16384 + b * 2048 + j),
            ):
                r0, ch = base // 3, base % 3
                XX[off : off + cnt] = xf[r0 : r0 + cnt]
                WW[:, j * 512 : (j + 1) * 512] = W_qkv[
                    :, ch * 1024 + hg * 512 : ch * 1024 + hg * 512 + 512
                ]
        per_core.append(
            dict(
                xq=np.ascontiguousarray(XQ.T).astype(bf16),
                xk=np.ascontiguousarray(XK.T).astype(bf16),
                xv=np.ascontiguousarray(XV.T).astype(bf16),
                wq=WQ.astype(bf16), wk=WK.astype(bf16), wv=WV.astype(bf16),
                wo=np.ascontiguousarray(W_out[hg * 512 : (hg + 1) * 512]).astype(
                    bf16
                ),
            )
        )
    return per_core


_PI = np.concatenate([np.arange(j, 2048, 3) for j in range(3)])


def unshard(core_outs, b_out):
    b_out = np.asarray(b_out, dtype=np.float32)
    out = np.empty((B, T, D), np.float32)
    for b in range(B):
        part = np.asarray(core_outs[2 * b], np.float32) + np.asarray(
            core_outs[2 * b + 1], np.float32
        )
        tmp = np.empty_like(part)
        tmp[_PI] = part
        out[b] = tmp + b_out
    return out


def kernel(x, W_qkv, b_qkv, W_out, b_out, num_heads):
    assert int(num_heads) == 16
    nc = get_nc(1)
    in_maps = shard_inputs(x, W_qkv, W_out)
    res = run_bass_kernel_spmd(nc, in_maps, core_ids=list(range(8)))
    return unshard([r["out"] for r in res.results], b_out)


# revision 20
# speedup vs baseline: 1.0408x; 1.0122x over previous
"""Fused full-attention kernel for Trainium2, SPMD over 8 NeuronCores.

Problem: nn_CausalSelfAttention (B=4, T=2048, D=1024, H=16, head_dim=64),
with the module's faithful-to-torch raw `.view(3,B,T,D)` reinterpretation of
the (B,T,3D) QKV projection buffer (NOT a feature-dim chunk), full (non-causal)
softmax over keys.

Sharding: core c handles batch b=c//2 and head-group hg=c%2 (8 heads). The raw
view means q/k/v token rows map to proj rows n//3 with column-chunk n%3; tokens
are processed in residue-class order (t mod 3), which makes every extraction a
contiguous slice. The host pre-permutes W_qkv columns per (b,hg,class) and
slices x rows per class, so one canonical SPMD program serves all cores. The
final output projection is computed per-core on the head-group's 512 columns;
host sums the two partial outputs per batch, un-permutes rows, and adds b_out.

v3: all operands bf16, uniform 16x128 token tiles, ones-column softmax
denominator, software-pipelined attention inner loop (QK of tile g+1 is
emitted before AV of tile g so the PE never blocks on the exp), and
tile-granular injection of the q-projection (later classes) and the output
projection into the ACT-bound attention phase.

NOTE: b_qkv is compiled in as zero (the problem spec fixes fill=zeros for it).
"""

import numpy as np
import ml_dtypes

import concourse.mybir as mybir
from concourse import bacc
from concourse.bass_utils import run_bass_kernel_spmd
from concourse.tile import TileContext

F32 = mybir.dt.float32
BF16 = mybir.dt.bfloat16
Exp = mybir.ActivationFunctionType.Exp

B, T, D = 4, 2048, 1024
CNT = (683, 683, 682)  # tokens per residue class (t % 3 == j)
OFF = (0, 683, 1366)
NG = 16  # uniform 128-token tiles over the grouped token axis


def build(reps: int = 1, stage: int = 4):
    nc = bacc.Bacc("TRN2", target_bir_lowering=False, debug=False)

    xq = nc.dram_tensor("xq", (D, 2048), BF16, kind="ExternalInput")
    xk = nc.dram_tensor("xk", (D, 2048), BF16, kind="ExternalInput")
    xv = nc.dram_tensor("xv", (D, 2048), BF16, kind="ExternalInput")
    wq = nc.dram_tensor("wq", (D, 1536), BF16, kind="ExternalInput")
    wk = nc.dram_tensor("wk", (D, 1536), BF16, kind="ExternalInput")
    wv = nc.dram_tensor("wv", (D, 1536), BF16, kind="ExternalInput")
    wo = nc.dram_tensor("wo", (512, 1024), BF16, kind="ExternalInput")
    out = nc.dram_tensor("out", (2048, 1024), BF16, kind="ExternalOutput")

    with TileContext(nc) as tc:
        with (
            tc.tile_pool(name="pers", bufs=1) as pers,
            tc.tile_pool(name="g1x", bufs=26) as xp,
            tc.tile_pool(name="g1w", bufs=9) as wp,
            tc.tile_pool(name="g1wv", bufs=10) as wvp,
            tc.tile_pool(name="mm", bufs=2, space="PSUM") as mmp,
            tc.tile_pool(name="att_st", bufs=2, space="PSUM") as stp,
            tc.tile_pool(name="att_in", bufs=1, space="PSUM") as inp,
            tc.tile_pool(name="att_ex", bufs=8) as exp_,
            tc.tile_pool(name="att_sm", bufs=2) as smp,
            tc.tile_pool(name="op_o", bufs=3) as otp,
        ):
            qTs = [pers.tile([128, 2048], BF16, tag=f"qT{i}", name=f"qT{i}") for i in range(4)]
            kTs = [pers.tile([128, 2048], BF16, tag=f"kT{i}", name=f"kT{i}") for i in range(4)]
            vs = [pers.tile([128, 520], BF16, tag=f"v{g}", name=f"v{g}") for g in range(NG)]
            inTs = [pers.tile([128, 2048], BF16, tag=f"inT{i}", name=f"inT{i}") for i in range(4)]
            wos = [pers.tile([128, 1024], BF16, tag=f"wo{i}", name=f"wo{i}") for i in range(4)]

            # ones columns of v (softmax denominator trick): written once,
            # disjoint from the per-rep data columns.
            for g in range(NG):
                vr = vs[g].rearrange("p (h e) -> p h e", e=65)
                nc.vector.memset(vr[:, :, 64:65], 1.0)

            def load_x(xd, j, eng=None):
                eng = eng or nc.scalar
                xts = []
                for d in range(8):
                    xt = xp.tile([128, 704], BF16, tag="x", name="xt")
                    eng.dma_start(
                        xt[:, 0 : CNT[j]],
                        xd[d * 128 : (d + 1) * 128, OFF[j] : OFF[j] + CNT[j]],
                    )
                    xts.append(xt)
                return xts

            def qk_chunk(xts, wt, dst, j, fp, a0, an):
                ps = mmp.tile([128, 512], F32, tag="mm", name="ps")
                for d in range(8):
                    nc.tensor.matmul(
                        ps[:, 0:an],
                        wt[:, d * 128 : (d + 1) * 128],
                        xts[d][:, a0 : a0 + an],
                        start=(d == 0),
                        stop=(d == 7),
                    )
                nc.vector.tensor_copy(
                    dst[fp][:, OFF[j] + a0 : OFF[j] + a0 + an], ps[:, 0:an]
                )

            def load_w(wd, j, fp, eng=None):
                eng = eng or nc.sync
                wd_r = wd.rearrange("(dt p) c -> p dt c", p=128)
                wt = wp.tile([128, 1024], BF16, tag="w", name="wt")
                c0 = j * 512 + fp * 128
                eng.dma_start(
                    wt[:].rearrange("p (dt c) -> p dt c", c=128),
                    wd_r[:, :, c0 : c0 + 128],
                )
                return wt

            def chunks_of(j):
                return (
                    ((0, 384), (CNT[j] - 300, 300))
                    if CNT[j] % 2
                    else ((0, 384), (384, CNT[j] - 384))
                )

            def emit_k():
                xts_all = [load_x(xk, j) for j in range(3)]
                wts_all = [[load_w(wk, j, fp) for fp in range(4)] for j in range(2)]
                for j in range(3):
                    for fp in range(4):
                        wt = wts_all[j][fp] if j < 2 else load_w(wk, j, fp)
                        for a0, an in chunks_of(j):
                            qk_chunk(xts_all[j], wt, kTs, j, fp, a0, an)

            def emit_v():
                # natural layout [tok, f]; realigned into uniform 128-token
                # vs tiles (interleaved 65-stride with the ones columns) by
                # partition-shifting DMAs from an SBUF staging tile.
                for j in range(3):
                    xts = load_x(xv, j)
                    wvts = []
                    for d in range(8):
                        wvt = wvp.tile([128, 512], BF16, tag="wv", name="wvt")
                        nc.scalar.dma_start(
                            wvt[:],
                            wv[d * 128 : (d + 1) * 128, j * 512 : (j + 1) * 512],
                        )
                        wvts.append(wvt)
                    for i0 in range(0, CNT[j], 128):
                        tp = min(128, CNT[j] - i0)
                        ps = mmp.tile([128, 512], F32, tag="mm", name="ps")
                        for d in range(8):
                            nc.tensor.matmul(
                                ps[0:tp, :],
                                xts[d][:, i0 : i0 + tp],
                                wvts[d][:],
                                start=(d == 0),
                                stop=(d == 7),
                            )
                        vst = wvp.tile([128, 512], BF16, tag="vst", name="vst")
                        nc.vector.tensor_copy(vst[0:tp, :], ps[0:tp, :])
                        vsr = vst.rearrange("p (h e) -> p h e", e=64)
                        g0 = OFF[j] + i0
                        g, p0 = g0 // 128, g0 % 128
                        r1 = min(tp, 128 - p0)
                        vr = vs[g].rearrange("p (h e) -> p h e", e=65)
                        nc.sync.dma_start(vr[p0 : p0 + r1, :, 0:64], vsr[0:r1, :, :])
                        if tp > r1:
                            vr2 = vs[g + 1].rearrange("p (h e) -> p h e", e=65)
                            nc.sync.dma_start(
                                vr2[0 : tp - r1, :, 0:64], vsr[r1:tp, :, :]
                            )

            def q_pieces(j):
                # 8 small closures (fp x chunk) emitting the q-projection for
                # class j; x and w tiles are loaded lazily by the first use.
                state = {}

                def piece(fp, a0, an):
                    if "xts" not in state:
                        state["xts"] = load_x(xq, j, eng=nc.sync)
                    if fp not in state:
                        state[fp] = load_w(wq, j, fp, eng=nc.sync)
                    qk_chunk(state["xts"], state[fp], qTs, j, fp, a0, an)

                return [
                    (lambda fp=fp, a0=a0, an=an: piece(fp, a0, an))
                    for fp in range(4)
                    for a0, an in chunks_of(j)
                ]

            def load_wo():
                for i in range(4):
                    nc.sync.dma_start(wos[i][:], wo[i * 128 : (i + 1) * 128, :])

            def op_fused(s16):
                # one row-block of the output projection: each stationary
                # (inT slice) is loaded once and serves both 512-col halves,
                # halving the LDWEIGHTS traffic.
                ot = otp.tile([128, 1024], BF16, tag="ot", name="ot")
                psA = mmp.tile([128, 512], F32, tag="mm", name="psA")
                psB = mmp.tile([128, 512], F32, tag="mm", name="psB")
                for fp in range(4):
                    nc.tensor.matmul(
                        psA[:],
                        inTs[fp][:, s16 * 128 : (s16 + 1) * 128],
                        wos[fp][:, 0:512],
                        start=(fp == 0),
                        stop=(fp == 3),
                    )
                    nc.tensor.matmul(
                        psB[:],
                        inTs[fp][:, s16 * 128 : (s16 + 1) * 128],
                        wos[fp][:, 512:1024],
                        start=(fp == 0),
                        stop=(fp == 3),
                    )
                nc.vector.tensor_copy(ot[:, 0:512], psA[:])
                nc.vector.tensor_copy(ot[:, 512:1024], psB[:])
                oeng = nc.scalar if s16 % 2 else nc.sync
                oeng.dma_start(out[s16 * 128 : (s16 + 1) * 128, :], ot[:])

            def attn_segment(s, fp, inject):
                sc0 = s * 512
                hA, hB = 2 * fp, 2 * fp + 1

                def qk(g):
                    t0 = g * 128
                    st = stp.tile([128, 1024], F32, tag="st", name="st")
                    nc.tensor.matmul(
                        st[:, 0:512],
                        kTs[fp][0:64, t0 : t0 + 128],
                        qTs[fp][0:64, sc0 : sc0 + 512],
                        start=True, stop=True, tile_position=(0, 0),
                    )
                    nc.tensor.matmul(
                        st[:, 512:1024],
                        kTs[fp][64:128, t0 : t0 + 128],
                        qTs[fp][64:128, sc0 : sc0 + 512],
                        start=True, stop=True, tile_position=(64, 0),
                    )
                    return st

                if stage >= 3:
                    inA = inp.tile([65, 512], F32, tag="inA", name="inA")
                    inB = inp.tile([65, 512], F32, tag="inB", name="inB")
                st = qk(0)
                for g in range(NG):
                    if stage >= 2:
                        ex = exp_.tile([128, 1024], BF16, tag="ex", name="ex")
                        nc.scalar.activation(ex[:], st[:], Exp, scale=0.125)
                    if g + 1 < NG:
                        st = qk(g + 1)
                    if stage >= 3:
                        nc.tensor.matmul(
                            inA[:],
                            vs[g][:, hA * 65 : hA * 65 + 65],
                            ex[:, 0:512],
                            start=(g == 0), stop=(g == NG - 1),
                        )
                        nc.tensor.matmul(
                            inB[:],
                            vs[g][:, hB * 65 : hB * 65 + 65],
                            ex[:, 512:1024],
                            start=(g == 0), stop=(g == NG - 1),
                        )
                    cb = inject.get(g)
                    if cb is not None:
                        cb()
                if stage < 3:
                    return
                # copy PSUM accumulators to SBUF promptly (frees the banks for
                # the next segment), then normalize by the ones-row sums.
                sAB = smp.tile([65, 1024], F32, tag="sAB", name="sAB")
                nc.vector.tensor_copy(sAB[:, 0:512], inA[:])
                nc.vector.tensor_copy(sAB[:, 512:1024], inB[:])
                rec = smp.tile([1, 1024], F32, tag="rec", name="rec")
                nc.vector.reciprocal(rec[:], sAB[64:65, :])
                bc = smp.tile([64, 1024], F32, tag="bc", name="bc")
                nc.gpsimd.partition_broadcast(bc[:], rec[:])
                nc.vector.tensor_mul(
                    inTs[fp][0:64, sc0 : sc0 + 512], sAB[0:64, 0:512], bc[:, 0:512]
                )
                stB = smp.tile([64, 512], BF16, tag="stB", name="stB")
                nc.vector.tensor_mul(stB[:], sAB[0:64, 512:1024], bc[:, 512:1024])
                nc.sync.dma_start(inTs[fp][64:128, sc0 : sc0 + 512], stB[:])

            def body():
                if stage >= 4:
                    load_wo()
                emit_k()
                emit_v()
                for u in q_pieces(0):
                    u()
                if stage == 0:
                    for j in (1, 2):
                        for u in q_pieces(j):
                            u()
                    return

                # injection schedule: {(s, fp): {tile g: closure}}
                inject = {(s, fp): {} for s in range(4) for fp in range(4)}

                def spread(s, items, slots):
                    n = len(items) // 4
                    for fp in range(4):
                        part = items[fp * n : (fp + 1) * n]
                        for i, it in enumerate(part):
                            inject[(s, fp)][slots[i]] = it

                spread(0, q_pieces(1), (5, 11))
                spread(1, q_pieces(2), (5, 11))
                if stage >= 4:
                    # output-projection row blocks: 2 fused pieces per sblk2
                    # segment (q rows of sblk0+sblk1), 1 per sblk3 segment,
                    # remainder in the tail.
                    for fp in range(4):
                        inject[(2, fp)][4] = lambda i=fp: op_fused(i)
                        inject[(2, fp)][11] = lambda i=4 + fp: op_fused(i)
                        inject[(3, fp)][7] = lambda i=8 + fp: op_fused(i)

                for s in range(4):
                    for fp in range(4):
                        attn_segment(s, fp, inject[(s, fp)])
                if stage >= 4:
                    for s16 in range(12, 16):
                        op_fused(s16)

            if reps == 1:
                body()
            else:
                # device-side loop: one emitted copy of the body, executed
                # `reps` times — amplifies body time for host-side timing
                # without blowing up the instruction count.
                with tc.For_i(0, reps, 1):
                    body()

    nc.compile()
    return nc


_CACHE: dict = {}


def get_nc(reps: int = 1):
    if reps not in _CACHE:
        _CACHE[reps] = build(reps)
    return _CACHE[reps]


_STAGES = {"gemm1": 0, "qk": 1, "exp": 2, "av": 3, "full": 4}


def build_mode(reps: int, mode: str):
    return build(reps, _STAGES[mode])


def shard_inputs(x, W_qkv, W_out):
    bf16 = ml_dtypes.bfloat16
    xf = np.ascontiguousarray(np.asarray(x, dtype=np.float32)).reshape(B * T, D)
    W_qkv = np.asarray(W_qkv, dtype=np.float32)
    W_out = np.asarray(W_out, dtype=np.float32)
    per_core = []
    for c in range(8):
        b, hg = c // 2, c % 2
        XQ = np.zeros((2048, D), np.float32)
        XK = np.zeros((2048, D), np.float32)
        XV = np.zeros((2048, D), np.float32)
        WQ = np.zeros((D, 1536), np.float32)
        WK = np.zeros((D, 1536), np.float32)
        WV = np.zeros((D, 1536), np.float32)
        for j in range(3):
            cnt, off = CNT[j], OFF[j]
            for XX, WW, base in (
                (XQ, WQ, b * 2048 + j),
                (XK, WK, 8192 + b * 2048 + j),
                (XV, WV, 16384 + b * 2048 + j),
            ):
                r0, ch = base // 3, base % 3
                XX[off : off + cnt] = xf[r0 : r0 + cnt]
                WW[:, j * 512 : (j + 1) * 512] = W_qkv[
                    :, ch * 1024 + hg * 512 : ch * 1024 + hg * 512 + 512
                ]
        per_core.append(
            dict(
                xq=np.ascontiguousarray(XQ.T).astype(bf16),
                xk=np.ascontiguousarray(XK.T).astype(bf16),
                xv=np.ascontiguousarray(XV.T).astype(bf16),
                wq=WQ.astype(bf16), wk=WK.astype(bf16), wv=WV.astype(bf16),
                wo=np.ascontiguousarray(W_out[hg * 512 : (hg + 1) * 512]).astype(
                    bf16
                ),
            )
        )
    return per_core


_PI = np.concatenate([np.arange(j, 2048, 3) for j in range(3)])


def unshard(core_outs, b_out):
    b_out = np.asarray(b_out, dtype=np.float32)
    out = np.empty((B, T, D), np.float32)
    for b in range(B):
        part = np.asarray(core_outs[2 * b], np.float32) + np.asarray(
            core_outs[2 * b + 1], np.float32
        )
        tmp = np.empty_like(part)
        tmp[_PI] = part
        out[b] = tmp + b_out
    return out


def kernel(x, W_qkv, b_qkv, W_out, b_out, num_heads):
    assert int(num_heads) == 16
    nc = get_nc(1)
    in_maps = shard_inputs(x, W_qkv, W_out)
    res = run_bass_kernel_spmd(nc, in_maps, core_ids=list(range(8)))
    return unshard([r["out"] for r in res.results], b_out)
